# revision 12
# baseline (speedup 1.0000x reference)
"""MCR loss kernel for Trainium2 (8 NeuronCores).

Strategy (v3):
  - Shard batch T=16 -> 2 timesteps per core (data parallel, no collectives).
  - Per core: 6 feature planes (2 timesteps x 3 maps); part A = groups 0-3
    (partition = (g, c), 128 partitions), part B = groups 4-5 packed as
    (k, g', c) where k picks a 24-input-row strip, so B reduces also run
    at full 128-partition width with contiguous 18.4KB DMA lines.
  - 8x8 avg-pool (sum; 1/64 folded into conv weights) as a SINGLE
    vector-engine XY reduce per 24-row slab (1 elem/cycle, no 2nd stage).
  - Reflect-pad + 3x3 conv: engine copies build a dy-replicated padded
    tile (fp32r-rounded), then 3 PE matmuls with K=(dy,ic)=96 in fp32r
    (single-pass, 2.3x faster than fp32); LeakyReLU via scalar PSUM copy
    + vector scalar_tensor_tensor max(0.2z, z).
  - B is streamed/processed first so its conv work completes while A is
    still streaming; the post-DMA tail is only the A path + Grams.
  - Gram G_t = V_t V_t^T via PE transpose + fp32r matmul over pixel chunks.
  - Host: matrix determinant lemma
        logdet(I_576 + a V^T V) = logdet(I_96 + a V V^T)
    so only the [2,96,96] Grams leave the device; float64 Cholesky logdets
    finish the scalar loss.
"""

import numpy as np

_STATE = {}

# -------- fixed problem geometry (hardcoded per harness contract) --------
B, CCH, H, W = 16, 32, 192, 192
NCORES = 8
TPC = B // NCORES          # timesteps per core = 2
OUT = 24                   # pooled spatial size
PIX = OUT * OUT            # 576
M = 96                     # feature rows (3 maps x 32 channels)
ALPHA_E = 6.0              # 576 / (96 * eps)
ALPHA_C = 18.0             # 576 / (32 * eps)


def _build_nc():
    import concourse.bass as bass
    import concourse.tile as tile
    from concourse import bacc, mybir

    DT = mybir.dt.float32
    DTR = mybir.dt.float32r
    nc = bacc.Bacc(
        "TRN2", target_bir_lowering=False, debug=False, num_devices=NCORES
    )

    # x[g] for g = t*3+m : feature-map plane stacks, host-reordered
    x = nc.declare_dram_parameter("x", [TPC * 3, CCH, H, W], DT, isOutput=False)
    wt = nc.declare_dram_parameter("wt", [3, 3, 96, 32], DT, isOutput=False)
    ident = nc.declare_dram_parameter("ident", [128, 128], DT, isOutput=False)
    g_out = nc.declare_dram_parameter("g_out", [TPC, M, M], DT, isOutput=True)

    with tile.TileContext(nc) as tc:
        with (
            tc.tile_pool(name="persist", bufs=1) as persist,
            tc.tile_pool(name="slabA", bufs=5) as slabA_pool,
            tc.tile_pool(name="slabB", bufs=4) as slabB_pool,
            tc.tile_pool(name="xrep", bufs=3) as xrep_pool,
            tc.tile_pool(name="zc", bufs=2) as zc_pool,
            tc.tile_pool(name="vt", bufs=3) as vt_pool,
            tc.tile_pool(name="psum", bufs=3, space="PSUM") as psum_pool,
            tc.tile_pool(name="psumt", bufs=2, space="PSUM") as psumt_pool,
            tc.tile_pool(name="psumg", bufs=1, space="PSUM") as psumg_pool,
        ):
            wt_sb = persist.tile([96, 288], DT, tag="wt")
            nc.gpsimd.dma_start(
                out=wt_sb[:].rearrange("p (m x c) -> p m x c", m=3, x=3),
                in_=wt.ap().rearrange("m x p c -> p m x c"),
            )
            id_sb = persist.tile([128, 128], DT, tag="ident")
            nc.gpsimd.dma_start(out=id_sb[:], in_=ident.ap())
            # fp32r-rounded copy of the weights (PE single-pass mode needs
            # its inputs produced as float32r)
            wt_r = persist.tile([96, 288], DTR, tag="wt_r")
            nc.scalar.copy(wt_r[:], wt_sb[:])

            # pooled layouts:
            #   A: partition (g, c), g=0..3; col = y*24 + x
            #   B: partition (k, g', c) = k*64 + g'*32 + c;
            #      col = i*72 + yq*24 + x  for global y = 6i + 3k + yq
            pooledA = persist.tile([128, PIX], DT, tag="pooledA")
            pooledB = persist.tile([128, 288], DT, tag="pooledB")
            v_sb = persist.tile([96, TPC * PIX], DT, tag="v")
            g_sb = persist.tile([96, TPC * 96], DT, tag="g")

            def reduce_slab(slab, out3):
                nc.vector.tensor_reduce(
                    out=out3,
                    in_=slab[:].rearrange(
                        "p (y r x w) -> p y x r w", y=3, r=8, x=24, w=8
                    ),
                    axis=mybir.AxisListType.XY,
                    op=mybir.AluOpType.add,
                )

            # ---- pooling: B slab i covers input rows 48i..48i+47 (k strips
            #      on separate DMA rings so both SDMA engine halves stay
            #      busy); A slab j covers input rows 24j..24j+23 ----
            for i in range(4):
                slabB = slabB_pool.tile([128, 24 * W], DT, tag="slabB")
                for k, ring in ((0, nc.scalar), (1, nc.gpsimd)):
                    rows = slice(48 * i + 24 * k, 48 * i + 24 * k + 24)
                    ring.dma_start(
                        out=slabB[64 * k : 64 * k + 64, :].rearrange(
                            "p (h w) -> p h w", h=24
                        ),
                        in_=x.ap()[4:6, :, rows, :].rearrange(
                            "g c h w -> (g c) h w"
                        ),
                    )
                reduce_slab(
                    slabB,
                    pooledB[:, i * 72 : (i + 1) * 72].rearrange(
                        "p (y x) -> p y x", y=3
                    ),
                )
            for j in range(8):
                rows = slice(24 * j, 24 * j + 24)
                slabA = slabA_pool.tile([128, 24 * W], DT, tag="slabA")
                nc.sync.dma_start(
                    out=slabA[:],
                    in_=x.ap()[0:4, :, rows, :].rearrange(
                        "g c h w -> (g c) (h w)"
                    ),
                )
                reduce_slab(
                    slabA,
                    pooledA[:, j * 72 : (j + 1) * 72].rearrange(
                        "p (y x) -> p y x", y=3
                    ),
                )

            # ---- conv helper: 3 dx matmuls + LeakyReLU into v_sb ----
            def conv_group(t, m, xr3):
                for half in range(2):
                    pc = psum_pool.tile([32, 288], DT, tag="convps")
                    for dx in range(3):
                        nc.tensor.matmul(
                            pc[:],
                            wt_r[:, (m * 3 + dx) * 32 : (m * 3 + dx + 1) * 32],
                            xr3[:, 12 * half : 12 * half + 12, dx : dx + 24],
                            start=(dx == 0),
                            stop=(dx == 2),
                        )
                    # LeakyReLU(0.2) == max(0.2*z, z); PSUM may feed only one
                    # non-scalar input, so stage a copy through SBUF first
                    zc = zc_pool.tile([32, 288], DT, tag="zcopy")
                    nc.scalar.copy(zc[:], pc[:])
                    nc.vector.scalar_tensor_tensor(
                        out=v_sb[
                            m * 32 : (m + 1) * 32,
                            t * PIX + half * 288 : t * PIX + (half + 1) * 288,
                        ],
                        in0=zc[:],
                        scalar=0.2,
                        in1=pc[:],
                        op0=mybir.AluOpType.mult,
                        op1=mybir.AluOpType.max,
                    )

            # ---- B-group convs (gi = 4, 5): processed first, mid-stream ----
            # xrep rows: dst y = y' + 1 - dy for source row y' = 6i + 3k + yq.
            # With xr6 = xrep viewed [p, yb(4), y6(6), xx(26)], dst y =
            # 6i + (yq + off), off = 3k + 1 - dy in {-1..4}: offsets 0..3 stay
            # inside a y6 block (one copy); -1 / 4 split into two copies.
            for gB in range(2):
                t, m = divmod(4 + gB, 3)
                xrep = xrep_pool.tile([96, 24 * 26], DTR, tag="xrep")
                xr3 = xrep[:].rearrange("p (y x) -> p y x", y=OUT)
                for dy in range(3):
                    dst6 = xr3[dy * 32 : (dy + 1) * 32].rearrange(
                        "p (i y6) x -> p i y6 x", i=4
                    )
                    for k in range(2):
                        srcB = pooledB[
                            k * 64 + gB * 32 : k * 64 + gB * 32 + 32, :
                        ].rearrange("p (i yq x) -> p i yq x", i=4, yq=3)
                        off = 3 * k + 1 - dy
                        if 0 <= off <= 3:
                            nc.scalar.copy(
                                dst6[:, :, off : off + 3, 1:25], srcB[:]
                            )
                        elif off == 4:
                            nc.scalar.copy(
                                dst6[:, :, 4:6, 1:25], srcB[:, :, 0:2, :]
                            )
                            nc.scalar.copy(
                                dst6[:, 1:4, 0:1, 1:25], srcB[:, 0:3, 2:3, :]
                            )
                        else:  # off == -1
                            nc.scalar.copy(
                                dst6[:, :, 0:2, 1:25], srcB[:, :, 1:3, :]
                            )
                            nc.scalar.copy(
                                dst6[:, 0:3, 5:6, 1:25], srcB[:, 1:4, 0:1, :]
                            )
                    # reflect rows: dy=0 -> dst y0 <- y'=1 (k=0, i=0, yq=1);
                    #               dy=2 -> dst y23 <- y'=22 (k=1, i=3, yq=1)
                    if dy == 0:
                        nc.scalar.copy(
                            xr3[dy * 32 : (dy + 1) * 32, 0:1, 1:25],
                            pooledB[gB * 32 : gB * 32 + 32, 24:48],
                        )
                    if dy == 2:
                        nc.scalar.copy(
                            xr3[dy * 32 : (dy + 1) * 32, 23:24, 1:25],
                            pooledB[64 + gB * 32 : 64 + gB * 32 + 32, 240:264],
                        )
                nc.scalar.copy(xr3[:, :, 0:1], xr3[:, :, 2:3])
                nc.scalar.copy(xr3[:, :, 25:26], xr3[:, :, 23:24])
                conv_group(t, m, xr3)

            # ---- A-group convs (tail): xrep copies split vector/scalar;
            #      gi=3 first so gram t1 can start while gi 0-2 still run ----
            for gi in (3, 0, 1, 2):
                t, m = divmod(gi, 3)
                xrep = xrep_pool.tile([96, 24 * 26], DTR, tag="xrep")
                xr3 = xrep[:].rearrange("p (y x) -> p y x", y=OUT)
                srcA = pooledA[gi * 32 : gi * 32 + 32, :].rearrange(
                    "p (y x) -> p y x", y=OUT
                )
                cp = (
                    nc.vector.tensor_copy if gi % 2 == 0 else nc.scalar.copy
                )
                for dy in range(3):
                    dst = xr3[dy * 32 : (dy + 1) * 32]
                    y0, y1 = max(0, 1 - dy), min(24, 25 - dy)
                    cp(dst[:, y0:y1, 1:25], srcA[:, y0 + dy - 1 : y1 + dy - 1, :])
                    if dy == 0:
                        cp(dst[:, 0:1, 1:25], srcA[:, 1:2, :])
                    if dy == 2:
                        cp(dst[:, 23:24, 1:25], srcA[:, 22:23, :])
                cp(xr3[:, :, 0:1], xr3[:, :, 2:3])
                cp(xr3[:, :, 25:26], xr3[:, :, 23:24])
                conv_group(t, m, xr3)

            # ---- Gram per t: transpose V chunks, then accumulate VT^T@VT.
            #      t1 first (its convs finish first) and chunk-interleaved
            #      so the transpose/copy/matmul chains pipeline ----
            gps = []
            for ti in range(TPC):
                gp = psumg_pool.tile([96, 96], DT, tag=f"gram{ti}")
                gps.append(gp)
            for c in range(5):
                sz = 128 if c < 4 else 64
                for t in (1, 0):
                    vslice = v_sb[:, t * PIX + c * 128 : t * PIX + c * 128 + sz]
                    pt = psumt_pool.tile([128, 96], DT, tag="vtps")
                    nc.tensor.transpose(pt[:sz, :], vslice, id_sb[:96, :96])
                    vt = vt_pool.tile([128, 96], DTR, tag="vt")
                    nc.scalar.copy(vt[:sz, :], pt[:sz, :])
                    nc.tensor.matmul(
                        gps[t][:], vt[:sz, :], vt[:sz, :],
                        start=(c == 0), stop=(c == 4),
                    )
            for t in (1, 0):
                nc.scalar.copy(g_sb[:, t * 96 : (t + 1) * 96], gps[t][:])
                nc.gpsimd.dma_start(
                    out=g_out[t], in_=g_sb[:, t * 96 : (t + 1) * 96]
                )

    nc.finalize()
    return nc


def _get_nc():
    if "nc" not in _STATE:
        _STATE["nc"] = _build_nc()
    return _STATE["nc"]


def _prep_weights(W1, W2, W3):
    # wt[m, dx, dy*32+ic, oc] = W_m[oc, ic, dy, dx] / 64   (pool-mean folded in)
    wt = np.stack(
        [np.asarray(w, np.float64).transpose(3, 2, 1, 0).reshape(3, 96, 32)
         for w in (W1, W2, W3)]
    ) / 64.0
    return np.ascontiguousarray(wt, dtype=np.float32)


def _host_loss(G):
    G = np.asarray(G, np.float64)  # [16, 96, 96]
    T = G.shape[0]
    I96 = np.eye(M)
    Me = I96[None] + ALPHA_E * G
    ld_e = 2.0 * np.log(
        np.diagonal(np.linalg.cholesky(Me), axis1=-2, axis2=-1)
    ).sum()
    blocks = np.stack(
        [G[:, 32 * c : 32 * (c + 1), 32 * c : 32 * (c + 1)] for c in range(3)]
    )  # [3, T, 32, 32]
    Mc = np.eye(32)[None, None] + ALPHA_C * blocks
    ld_c = 2.0 * np.log(
        np.diagonal(np.linalg.cholesky(Mc), axis1=-2, axis2=-1)
    ).sum()
    loss_expd = ld_e / (2.0 * T)
    loss_comp = (32.0 / M) * ld_c / (2.0 * T)
    return np.float32(loss_expd - loss_comp)


def run_device(inputs, **kw):
    """Run the bass kernel; returns (G [16,96,96], BassKernelResults)."""
    from concourse.bass_utils import run_bass_kernel_spmd

    nc = _get_nc()
    wt = _prep_weights(inputs["W1"], inputs["W2"], inputs["W3"])
    ident = np.eye(128, dtype=np.float32)
    ms = np.asarray(inputs["ms_fea"], np.float32)
    pan = np.asarray(inputs["pan_fea"], np.float32)
    alf = np.asarray(inputs["all_fea"], np.float32)
    in_maps = []
    for i in range(NCORES):
        sl = slice(TPC * i, TPC * (i + 1))
        # x[t*3+m] = (ms,pan,alf)[m][t]
        xs = np.stack([ms[sl], pan[sl], alf[sl]], axis=1).reshape(
            TPC * 3, CCH, H, W
        )
        in_maps.append(
            {"x": np.ascontiguousarray(xs), "wt": wt, "ident": ident}
        )
    res = run_bass_kernel_spmd(nc, in_maps, core_ids=list(range(NCORES)), **kw)
    G = np.concatenate([np.asarray(r["g_out"]) for r in res.results], axis=0)
    return G, res


def kernel(**inputs):
    G, _ = run_device(inputs)
    return _host_loss(G)


# revision 16
# speedup vs baseline: 1.1245x; 1.1245x over previous
"""MCR loss kernel for Trainium2 (8 NeuronCores).

Strategy (v3):
  - Shard batch T=16 -> 2 timesteps per core (data parallel, no collectives).
  - Per core: 6 feature planes (2 timesteps x 3 maps); part A = groups 0-3
    (partition = (g, c), 128 partitions), part B = groups 4-5 packed as
    (k, g', c) where k picks a 24-input-row strip, so B reduces also run
    at full 128-partition width with contiguous 18.4KB DMA lines.
  - 8x8 avg-pool (sum; 1/64 folded into conv weights) as a SINGLE
    vector-engine XY reduce per 24-row slab (1 elem/cycle, no 2nd stage).
  - Reflect-pad + 3x3 conv: engine copies build a dy-replicated padded
    tile (fp32r-rounded), then 3 PE matmuls with K=(dy,ic)=96 in fp32r
    (single-pass, 2.3x faster than fp32); LeakyReLU via scalar PSUM copy
    + vector scalar_tensor_tensor max(0.2z, z).
  - B is streamed/processed first so its conv work completes while A is
    still streaming; the post-DMA tail is only the A path + Grams.
  - Gram G_t = V_t V_t^T via PE transpose + fp32r matmul over pixel chunks.
  - Host: matrix determinant lemma
        logdet(I_576 + a V^T V) = logdet(I_96 + a V V^T)
    so only the [2,96,96] Grams leave the device; float64 Cholesky logdets
    finish the scalar loss.
"""

import numpy as np

_STATE = {}

# -------- fixed problem geometry (hardcoded per harness contract) --------
B, CCH, H, W = 16, 32, 192, 192
NCORES = 8
TPC = B // NCORES          # timesteps per core = 2
OUT = 24                   # pooled spatial size
PIX = OUT * OUT            # 576
M = 96                     # feature rows (3 maps x 32 channels)
ALPHA_E = 6.0              # 576 / (96 * eps)
ALPHA_C = 18.0             # 576 / (32 * eps)


def _build_nc():
    import concourse.bass as bass
    import concourse.tile as tile
    from concourse import bacc, mybir

    DT = mybir.dt.float32
    DTR = mybir.dt.float32r
    nc = bacc.Bacc(
        "TRN2", target_bir_lowering=False, debug=False, num_devices=NCORES
    )

    # x[g] for g = t*3+m : feature-map plane stacks, host-reordered
    x = nc.declare_dram_parameter("x", [TPC * 3, CCH, H, W], DT, isOutput=False)
    wt = nc.declare_dram_parameter("wt", [3, 3, 96, 32], DT, isOutput=False)
    ident = nc.declare_dram_parameter("ident", [128, 128], DT, isOutput=False)
    g_out = nc.declare_dram_parameter("g_out", [TPC, M, M], DT, isOutput=True)

    with tile.TileContext(nc) as tc:
        with (
            tc.tile_pool(name="persist", bufs=1) as persist,
            tc.tile_pool(name="slabA", bufs=2) as slabA_pool,
            tc.tile_pool(name="slabA24", bufs=2) as slabA24_pool,
            tc.tile_pool(name="slabB", bufs=2) as slabB_pool,
            tc.tile_pool(name="xrep", bufs=3) as xrep_pool,
            tc.tile_pool(name="zc", bufs=2) as zc_pool,
            tc.tile_pool(name="vt", bufs=3) as vt_pool,
            tc.tile_pool(name="psum", bufs=3, space="PSUM") as psum_pool,
            tc.tile_pool(name="psumt", bufs=2, space="PSUM") as psumt_pool,
            tc.tile_pool(name="psumg", bufs=1, space="PSUM") as psumg_pool,
        ):
            wt_sb = persist.tile([96, 288], DT, tag="wt")
            nc.gpsimd.dma_start(
                out=wt_sb[:].rearrange("p (m x c) -> p m x c", m=3, x=3),
                in_=wt.ap().rearrange("m x p c -> p m x c"),
            )
            id_sb = persist.tile([128, 128], DT, tag="ident")
            nc.gpsimd.dma_start(out=id_sb[:], in_=ident.ap())
            # fp32r-rounded copy of the weights (PE single-pass mode needs
            # its inputs produced as float32r)
            wt_r = persist.tile([96, 288], DTR, tag="wt_r")
            nc.scalar.copy(wt_r[:], wt_sb[:])

            # pooled layouts:
            #   A: partition (g, c), g=0..3; col = y*24 + x
            #   B: partition (k, g', c) = k*64 + g'*32 + c;
            #      col = i*72 + yq*24 + x  for global y = 6i + 3k + yq
            pooledA = persist.tile([128, PIX], DT, tag="pooledA")
            pooledB = persist.tile([128, 288], DT, tag="pooledB")
            v_sb = persist.tile([96, TPC * PIX], DT, tag="v")
            g_sb = persist.tile([96, TPC * 96], DT, tag="g")

            def reduce_slab(slab, out3, y):
                nc.vector.tensor_reduce(
                    out=out3,
                    in_=slab[:].rearrange(
                        "p (y r x w) -> p y x r w", y=y, r=8, x=24, w=8
                    ),
                    axis=mybir.AxisListType.XY,
                    op=mybir.AluOpType.add,
                )

            # ---- pooling. ALL input DMA on the single sync HWDGE ring in
            # need order: multiple concurrent queues interleave packets and
            # halve HBM efficiency (measured 217 vs 424 GB/s).
            #   B slab i covers input rows 48i..48i+47 as two 24-row k
            #   strips; A = three 48-row slabs + two 24-row slabs (smaller
            #   final reduces shorten the post-stream tail). ----
            def dma_B(i):
                slabB = slabB_pool.tile([128, 24 * W], DT, tag="slabB")
                for k in range(2):
                    rows = slice(48 * i + 24 * k, 48 * i + 24 * k + 24)
                    nc.sync.dma_start(
                        out=slabB[64 * k : 64 * k + 64, :].rearrange(
                            "p (h w) -> p h w", h=24
                        ),
                        in_=x.ap()[4:6, :, rows, :].rearrange(
                            "g c h w -> (g c) h w"
                        ),
                    )
                reduce_slab(
                    slabB,
                    pooledB[:, i * 72 : (i + 1) * 72].rearrange(
                        "p (y x) -> p y x", y=3
                    ),
                    y=3,
                )

            def dma_A(q, nrows):
                rows = slice(24 * q, 24 * q + nrows)
                pool = slabA_pool if nrows == 48 else slabA24_pool
                slabA = pool.tile([128, nrows * W], DT, tag=f"slabA{nrows}")
                nc.sync.dma_start(
                    out=slabA[:],
                    in_=x.ap()[0:4, :, rows, :].rearrange(
                        "g c h w -> (g c) (h w)"
                    ),
                )
                yy = nrows // 8
                reduce_slab(
                    slabA,
                    pooledA[:, q * 72 : q * 72 + yy * 24].rearrange(
                        "p (y x) -> p y x", y=yy
                    ),
                    y=yy,
                )

            dma_B(0)
            dma_A(0, 48)
            dma_B(1)
            dma_A(2, 48)
            dma_B(2)
            dma_A(4, 48)
            dma_B(3)
            dma_A(6, 24)
            dma_A(7, 24)

            # ---- conv helper: 3 dx matmuls + LeakyReLU into v_sb ----
            def conv_group(t, m, xr3):
                for half in range(2):
                    pc = psum_pool.tile([32, 288], DT, tag="convps")
                    for dx in range(3):
                        nc.tensor.matmul(
                            pc[:],
                            wt_r[:, (m * 3 + dx) * 32 : (m * 3 + dx + 1) * 32],
                            xr3[:, 12 * half : 12 * half + 12, dx : dx + 24],
                            start=(dx == 0),
                            stop=(dx == 2),
                        )
                    # LeakyReLU(0.2) == max(0.2*z, z); PSUM may feed only one
                    # non-scalar input, so stage a copy through SBUF first
                    zc = zc_pool.tile([32, 288], DT, tag="zcopy")
                    nc.scalar.copy(zc[:], pc[:])
                    nc.vector.scalar_tensor_tensor(
                        out=v_sb[
                            m * 32 : (m + 1) * 32,
                            t * PIX + half * 288 : t * PIX + (half + 1) * 288,
                        ],
                        in0=zc[:],
                        scalar=0.2,
                        in1=pc[:],
                        op0=mybir.AluOpType.mult,
                        op1=mybir.AluOpType.max,
                    )

            # ---- B-group convs (gi = 4, 5): processed first, mid-stream ----
            # xrep rows: dst y = y' + 1 - dy for source row y' = 6i + 3k + yq.
            # With xr6 = xrep viewed [p, yb(4), y6(6), xx(26)], dst y =
            # 6i + (yq + off), off = 3k + 1 - dy in {-1..4}: offsets 0..3 stay
            # inside a y6 block (one copy); -1 / 4 split into two copies.
            for gB in range(2):
                t, m = divmod(4 + gB, 3)
                xrep = xrep_pool.tile([96, 24 * 26], DTR, tag="xrep")
                xr3 = xrep[:].rearrange("p (y x) -> p y x", y=OUT)
                for dy in range(3):
                    dst6 = xr3[dy * 32 : (dy + 1) * 32].rearrange(
                        "p (i y6) x -> p i y6 x", i=4
                    )
                    for k in range(2):
                        srcB = pooledB[
                            k * 64 + gB * 32 : k * 64 + gB * 32 + 32, :
                        ].rearrange("p (i yq x) -> p i yq x", i=4, yq=3)
                        off = 3 * k + 1 - dy
                        if 0 <= off <= 3:
                            nc.scalar.copy(
                                dst6[:, :, off : off + 3, 1:25], srcB[:]
                            )
                        elif off == 4:
                            nc.scalar.copy(
                                dst6[:, :, 4:6, 1:25], srcB[:, :, 0:2, :]
                            )
                            nc.scalar.copy(
                                dst6[:, 1:4, 0:1, 1:25], srcB[:, 0:3, 2:3, :]
                            )
                        else:  # off == -1
                            nc.scalar.copy(
                                dst6[:, :, 0:2, 1:25], srcB[:, :, 1:3, :]
                            )
                            nc.scalar.copy(
                                dst6[:, 0:3, 5:6, 1:25], srcB[:, 1:4, 0:1, :]
                            )
                    # reflect rows: dy=0 -> dst y0 <- y'=1 (k=0, i=0, yq=1);
                    #               dy=2 -> dst y23 <- y'=22 (k=1, i=3, yq=1)
                    if dy == 0:
                        nc.scalar.copy(
                            xr3[dy * 32 : (dy + 1) * 32, 0:1, 1:25],
                            pooledB[gB * 32 : gB * 32 + 32, 24:48],
                        )
                    if dy == 2:
                        nc.scalar.copy(
                            xr3[dy * 32 : (dy + 1) * 32, 23:24, 1:25],
                            pooledB[64 + gB * 32 : 64 + gB * 32 + 32, 240:264],
                        )
                nc.scalar.copy(xr3[:, :, 0:1], xr3[:, :, 2:3])
                nc.scalar.copy(xr3[:, :, 25:26], xr3[:, :, 23:24])
                conv_group(t, m, xr3)

            # ---- A-group convs (tail): gi=3 first so gram t1 can start
            #      while gi 0-2 still run ----
            for gi in (3, 0, 1, 2):
                t, m = divmod(gi, 3)
                xrep = xrep_pool.tile([96, 24 * 26], DTR, tag="xrep")
                xr3 = xrep[:].rearrange("p (y x) -> p y x", y=OUT)
                srcA = pooledA[gi * 32 : gi * 32 + 32, :].rearrange(
                    "p (y x) -> p y x", y=OUT
                )
                cp = nc.scalar.copy
                for dy in range(3):
                    dst = xr3[dy * 32 : (dy + 1) * 32]
                    y0, y1 = max(0, 1 - dy), min(24, 25 - dy)
                    cp(dst[:, y0:y1, 1:25], srcA[:, y0 + dy - 1 : y1 + dy - 1, :])
                    if dy == 0:
                        cp(dst[:, 0:1, 1:25], srcA[:, 1:2, :])
                    if dy == 2:
                        cp(dst[:, 23:24, 1:25], srcA[:, 22:23, :])
                cp(xr3[:, :, 0:1], xr3[:, :, 2:3])
                cp(xr3[:, :, 25:26], xr3[:, :, 23:24])
                conv_group(t, m, xr3)

            # ---- Gram per t: transpose V chunks, then accumulate VT^T@VT.
            #      t1 first (its convs finish first) and chunk-interleaved
            #      so the transpose/copy/matmul chains pipeline ----
            gps = []
            for ti in range(TPC):
                gp = psumg_pool.tile([96, 96], DT, tag=f"gram{ti}")
                gps.append(gp)
            for c in range(5):
                sz = 128 if c < 4 else 64
                for t in (1, 0):
                    vslice = v_sb[:, t * PIX + c * 128 : t * PIX + c * 128 + sz]
                    pt = psumt_pool.tile([128, 96], DT, tag="vtps")
                    nc.tensor.transpose(pt[:sz, :], vslice, id_sb[:96, :96])
                    vt = vt_pool.tile([128, 96], DTR, tag="vt")
                    nc.scalar.copy(vt[:sz, :], pt[:sz, :])
                    nc.tensor.matmul(
                        gps[t][:], vt[:sz, :], vt[:sz, :],
                        start=(c == 0), stop=(c == 4),
                    )
            for t in (1, 0):
                nc.scalar.copy(g_sb[:, t * 96 : (t + 1) * 96], gps[t][:])
                nc.gpsimd.dma_start(
                    out=g_out[t], in_=g_sb[:, t * 96 : (t + 1) * 96]
                )

    nc.finalize()
    return nc


def _get_nc():
    if "nc" not in _STATE:
        _STATE["nc"] = _build_nc()
    return _STATE["nc"]


def _prep_weights(W1, W2, W3):
    # wt[m, dx, dy*32+ic, oc] = W_m[oc, ic, dy, dx] / 64   (pool-mean folded in)
    wt = np.stack(
        [np.asarray(w, np.float64).transpose(3, 2, 1, 0).reshape(3, 96, 32)
         for w in (W1, W2, W3)]
    ) / 64.0
    return np.ascontiguousarray(wt, dtype=np.float32)


def _host_loss(G):
    G = np.asarray(G, np.float64)  # [16, 96, 96]
    T = G.shape[0]
    I96 = np.eye(M)
    Me = I96[None] + ALPHA_E * G
    ld_e = 2.0 * np.log(
        np.diagonal(np.linalg.cholesky(Me), axis1=-2, axis2=-1)
    ).sum()
    blocks = np.stack(
        [G[:, 32 * c : 32 * (c + 1), 32 * c : 32 * (c + 1)] for c in range(3)]
    )  # [3, T, 32, 32]
    Mc = np.eye(32)[None, None] + ALPHA_C * blocks
    ld_c = 2.0 * np.log(
        np.diagonal(np.linalg.cholesky(Mc), axis1=-2, axis2=-1)
    ).sum()
    loss_expd = ld_e / (2.0 * T)
    loss_comp = (32.0 / M) * ld_c / (2.0 * T)
    return np.float32(loss_expd - loss_comp)


def run_device(inputs, **kw):
    """Run the bass kernel; returns (G [16,96,96], BassKernelResults)."""
    from concourse.bass_utils import run_bass_kernel_spmd

    nc = _get_nc()
    wt = _prep_weights(inputs["W1"], inputs["W2"], inputs["W3"])
    ident = np.eye(128, dtype=np.float32)
    ms = np.asarray(inputs["ms_fea"], np.float32)
    pan = np.asarray(inputs["pan_fea"], np.float32)
    alf = np.asarray(inputs["all_fea"], np.float32)
    in_maps = []
    for i in range(NCORES):
        sl = slice(TPC * i, TPC * (i + 1))
        # x[t*3+m] = (ms,pan,alf)[m][t]
        xs = np.stack([ms[sl], pan[sl], alf[sl]], axis=1).reshape(
            TPC * 3, CCH, H, W
        )
        in_maps.append(
            {"x": np.ascontiguousarray(xs), "wt": wt, "ident": ident}
        )
    res = run_bass_kernel_spmd(nc, in_maps, core_ids=list(range(NCORES)), **kw)
    G = np.concatenate([np.asarray(r["g_out"]) for r in res.results], axis=0)
    return G, res


def kernel(**inputs):
    G, _ = run_device(inputs)
    return _host_loss(G)


# revision 25
# speedup vs baseline: 1.4099x; 1.2538x over previous
"""MCR loss kernel for Trainium2 (8 NeuronCores).

Strategy (v3):
  - Shard batch T=16 -> 2 timesteps per core (data parallel, no collectives).
  - Per core: 6 feature planes (2 timesteps x 3 maps); part A = groups 0-3
    (partition = (g, c), 128 partitions), part B = groups 4-5 packed as
    (k, g', c) where k picks a 24-input-row strip, so B reduces also run
    at full 128-partition width with contiguous 18.4KB DMA lines.
  - 8x8 avg-pool (sum; 1/64 folded into conv weights) as a SINGLE
    vector-engine XY reduce per 24-row slab (1 elem/cycle, no 2nd stage).
  - Reflect-pad + 3x3 conv: engine copies build a dy-replicated padded
    tile (fp32r-rounded), then 3 PE matmuls with K=(dy,ic)=96 in fp32r
    (single-pass, 2.3x faster than fp32); LeakyReLU via scalar PSUM copy
    + vector scalar_tensor_tensor max(0.2z, z).
  - B is streamed/processed first so its conv work completes while A is
    still streaming; the post-DMA tail is only the A path + Grams.
  - Gram G_t = V_t V_t^T via PE transpose + fp32r matmul over pixel chunks.
  - Host: matrix determinant lemma
        logdet(I_576 + a V^T V) = logdet(I_96 + a V V^T)
    so only the [2,96,96] Grams leave the device; float64 Cholesky logdets
    finish the scalar loss.
"""

import numpy as np

_STATE = {}

# -------- fixed problem geometry (hardcoded per harness contract) --------
B, CCH, H, W = 16, 32, 192, 192
NCORES = 8
TPC = B // NCORES          # timesteps per core = 2
OUT = 24                   # pooled spatial size
PIX = OUT * OUT            # 576
M = 96                     # feature rows (3 maps x 32 channels)
ALPHA_E = 6.0              # 576 / (96 * eps)
ALPHA_C = 18.0             # 576 / (32 * eps)


def _build_nc():
    import concourse.bass as bass
    import concourse.tile as tile
    from concourse import bacc, mybir

    DT = mybir.dt.float32
    DTR = mybir.dt.float32r
    nc = bacc.Bacc(
        "TRN2", target_bir_lowering=False, debug=False, num_devices=NCORES
    )

    # xa[g] for g = t*3+m, g<4 : feature-map plane stacks, host-reordered.
    # xb: B-part (groups 4,5) host-prearranged partition-major
    #     xb[k*64+g'*32+c, 24q+r, w] = plane[4+g'][c, 48q+24k+r, w]
    # so every B slab DMA is a full-width contiguous transfer.
    xa = nc.declare_dram_parameter("xa", [4, CCH, H, W], DT, isOutput=False)
    xb = nc.declare_dram_parameter("xb", [128, 96, W], DT, isOutput=False)
    wt = nc.declare_dram_parameter("wt", [3, 3, 96, 32], DT, isOutput=False)
    ident = nc.declare_dram_parameter("ident", [128, 128], DT, isOutput=False)
    g_out = nc.declare_dram_parameter("g_out", [TPC, M, M], DT, isOutput=True)

    with tile.TileContext(nc) as tc:
        with (
            tc.tile_pool(name="persist", bufs=1) as persist,
            tc.tile_pool(name="slabA", bufs=2) as slabA_pool,
            tc.tile_pool(name="slabA24", bufs=2) as slabA24_pool,
            tc.tile_pool(name="slabB", bufs=2) as slabB_pool,
            tc.tile_pool(name="xrep", bufs=3) as xrep_pool,
            tc.tile_pool(name="zc", bufs=2) as zc_pool,
            tc.tile_pool(name="vt", bufs=3) as vt_pool,
            tc.tile_pool(name="psum", bufs=3, space="PSUM") as psum_pool,
            tc.tile_pool(name="psumt", bufs=2, space="PSUM") as psumt_pool,
            tc.tile_pool(name="psumg", bufs=1, space="PSUM") as psumg_pool,
        ):
            wt_sb = persist.tile([96, 288], DT, tag="wt")
            nc.gpsimd.dma_start(
                out=wt_sb[:].rearrange("p (m x c) -> p m x c", m=3, x=3),
                in_=wt.ap().rearrange("m x p c -> p m x c"),
            )
            id_sb = persist.tile([128, 128], DT, tag="ident")
            nc.gpsimd.dma_start(out=id_sb[:], in_=ident.ap())
            # fp32r-rounded copy of the weights (PE single-pass mode needs
            # its inputs produced as float32r)
            wt_r = persist.tile([96, 288], DTR, tag="wt_r")
            nc.scalar.copy(wt_r[:], wt_sb[:])

            # pooled layouts:
            #   A: partition (g, c), g=0..3; col = y*24 + x
            #   B: partition (k, g', c) = k*64 + g'*32 + c;
            #      col = i*72 + yq*24 + x  for global y = 6i + 3k + yq
            pooledA = persist.tile([128, PIX], DT, tag="pooledA")
            pooledB = persist.tile([128, 288], DT, tag="pooledB")
            v_sb = persist.tile([96, TPC * PIX], DT, tag="v")
            g_sb = persist.tile([96, TPC * 96], DT, tag="g")

            def reduce_slab(slab, out3, y):
                nc.vector.tensor_reduce(
                    out=out3,
                    in_=slab[:].rearrange(
                        "p (y r x w) -> p y x r w", y=y, r=8, x=24, w=8
                    ),
                    axis=mybir.AxisListType.XY,
                    op=mybir.AluOpType.add,
                )

            # ---- pooling. ALL input DMA on the single sync HWDGE ring in
            # need order: multiple concurrent queues interleave packets and
            # halve HBM efficiency (measured 217 vs 424 GB/s).
            #   B slab i covers input rows 48i..48i+47 as two 24-row k
            #   strips; A = three 48-row slabs + two 24-row slabs (smaller
            #   final reduces shorten the post-stream tail). ----
            def dma_B(i):
                # one full-width contiguous DMA (xb host-prearranged)
                slabB = slabB_pool.tile([128, 24 * W], DT, tag="slabB")
                nc.sync.dma_start(
                    out=slabB[:],
                    in_=xb.ap()[:, 24 * i : 24 * i + 24, :].rearrange(
                        "p h w -> p (h w)"
                    ),
                )
                reduce_slab(
                    slabB,
                    pooledB[:, i * 72 : (i + 1) * 72].rearrange(
                        "p (y x) -> p y x", y=3
                    ),
                    y=3,
                )

            def dma_A(q, nrows):
                rows = slice(24 * q, 24 * q + nrows)
                pool = slabA_pool if nrows == 48 else slabA24_pool
                slabA = pool.tile([128, nrows * W], DT, tag=f"slabA{nrows}")
                nc.sync.dma_start(
                    out=slabA[:],
                    in_=xa.ap()[:, :, rows, :].rearrange(
                        "g c h w -> (g c) (h w)"
                    ),
                )
                yy = nrows // 8
                reduce_slab(
                    slabA,
                    pooledA[:, q * 72 : q * 72 + yy * 24].rearrange(
                        "p (y x) -> p y x", y=yy
                    ),
                    y=yy,
                )

            dma_B(0)
            dma_A(0, 48)
            dma_B(1)
            dma_A(2, 48)
            dma_B(2)
            dma_A(4, 48)
            dma_B(3)
            dma_A(6, 24)
            dma_A(7, 24)

            # ---- conv helper: 3 dx matmuls + LeakyReLU into v_sb ----
            def conv_group(t, m, xr3):
                for half in range(2):
                    pc = psum_pool.tile([32, 288], DT, tag="convps")
                    for dx in range(3):
                        nc.tensor.matmul(
                            pc[:],
                            wt_r[:, (m * 3 + dx) * 32 : (m * 3 + dx + 1) * 32],
                            xr3[:, 12 * half : 12 * half + 12, dx : dx + 24],
                            start=(dx == 0),
                            stop=(dx == 2),
                        )
                    # LeakyReLU(0.2) == max(0.2*z, z); PSUM may feed only one
                    # non-scalar input, so stage a copy through SBUF first
                    zc = zc_pool.tile([32, 288], DT, tag="zcopy")
                    nc.scalar.copy(zc[:], pc[:])
                    nc.vector.scalar_tensor_tensor(
                        out=v_sb[
                            m * 32 : (m + 1) * 32,
                            t * PIX + half * 288 : t * PIX + (half + 1) * 288,
                        ],
                        in0=zc[:],
                        scalar=0.2,
                        in1=pc[:],
                        op0=mybir.AluOpType.mult,
                        op1=mybir.AluOpType.max,
                    )

            # ---- B-group convs (gi = 4, 5): processed first, mid-stream ----
            # xrep rows: dst y = y' + 1 - dy for source row y' = 6i + 3k + yq.
            # With xr6 = xrep viewed [p, yb(4), y6(6), xx(26)], dst y =
            # 6i + (yq + off), off = 3k + 1 - dy in {-1..4}: offsets 0..3 stay
            # inside a y6 block (one copy); -1 / 4 split into two copies.
            for gB in range(2):
                t, m = divmod(4 + gB, 3)
                xrep = xrep_pool.tile([96, 24 * 26], DTR, tag="xrep")
                xr3 = xrep[:].rearrange("p (y x) -> p y x", y=OUT)
                for dy in range(3):
                    dst6 = xr3[dy * 32 : (dy + 1) * 32].rearrange(
                        "p (i y6) x -> p i y6 x", i=4
                    )
                    for k in range(2):
                        srcB = pooledB[
                            k * 64 + gB * 32 : k * 64 + gB * 32 + 32, :
                        ].rearrange("p (i yq x) -> p i yq x", i=4, yq=3)
                        off = 3 * k + 1 - dy
                        if 0 <= off <= 3:
                            nc.scalar.copy(
                                dst6[:, :, off : off + 3, 1:25], srcB[:]
                            )
                        elif off == 4:
                            nc.scalar.copy(
                                dst6[:, :, 4:6, 1:25], srcB[:, :, 0:2, :]
                            )
                            nc.scalar.copy(
                                dst6[:, 1:4, 0:1, 1:25], srcB[:, 0:3, 2:3, :]
                            )
                        else:  # off == -1
                            nc.scalar.copy(
                                dst6[:, :, 0:2, 1:25], srcB[:, :, 1:3, :]
                            )
                            nc.scalar.copy(
                                dst6[:, 0:3, 5:6, 1:25], srcB[:, 1:4, 0:1, :]
                            )
                    # reflect rows: dy=0 -> dst y0 <- y'=1 (k=0, i=0, yq=1);
                    #               dy=2 -> dst y23 <- y'=22 (k=1, i=3, yq=1)
                    if dy == 0:
                        nc.scalar.copy(
                            xr3[dy * 32 : (dy + 1) * 32, 0:1, 1:25],
                            pooledB[gB * 32 : gB * 32 + 32, 24:48],
                        )
                    if dy == 2:
                        nc.scalar.copy(
                            xr3[dy * 32 : (dy + 1) * 32, 23:24, 1:25],
                            pooledB[64 + gB * 32 : 64 + gB * 32 + 32, 240:264],
                        )
                nc.scalar.copy(xr3[:, :, 0:1], xr3[:, :, 2:3])
                nc.scalar.copy(xr3[:, :, 25:26], xr3[:, :, 23:24])
                conv_group(t, m, xr3)

            # ---- A-group convs (tail): gi=3 first so gram t1 can start
            #      while gi 0-2 still run ----
            for gi in (3, 0, 1, 2):
                t, m = divmod(gi, 3)
                xrep = xrep_pool.tile([96, 24 * 26], DTR, tag="xrep")
                xr3 = xrep[:].rearrange("p (y x) -> p y x", y=OUT)
                srcA = pooledA[gi * 32 : gi * 32 + 32, :].rearrange(
                    "p (y x) -> p y x", y=OUT
                )
                # vector is idle in the tail: split the xrep builds across
                # vector and scalar so the four groups finish ~2x sooner
                cp = nc.vector.tensor_copy if gi in (3, 1) else nc.scalar.copy
                for dy in range(3):
                    dst = xr3[dy * 32 : (dy + 1) * 32]
                    y0, y1 = max(0, 1 - dy), min(24, 25 - dy)
                    cp(dst[:, y0:y1, 1:25], srcA[:, y0 + dy - 1 : y1 + dy - 1, :])
                    if dy == 0:
                        cp(dst[:, 0:1, 1:25], srcA[:, 1:2, :])
                    if dy == 2:
                        cp(dst[:, 23:24, 1:25], srcA[:, 22:23, :])
                cp(xr3[:, :, 0:1], xr3[:, :, 2:3])
                cp(xr3[:, :, 25:26], xr3[:, :, 23:24])
                conv_group(t, m, xr3)

            # ---- Gram per t: transpose V chunks, then accumulate VT^T@VT.
            #      t1 first (its convs finish first) and chunk-interleaved
            #      so the transpose/copy/matmul chains pipeline ----
            gps = []
            for ti in range(TPC):
                gp = psumg_pool.tile([96, 96], DT, tag=f"gram{ti}")
                gps.append(gp)
            for c in range(5):
                sz = 128 if c < 4 else 64
                for t in (1, 0):
                    vslice = v_sb[:, t * PIX + c * 128 : t * PIX + c * 128 + sz]
                    pt = psumt_pool.tile([128, 96], DT, tag="vtps")
                    nc.tensor.transpose(pt[:sz, :], vslice, id_sb[:96, :96])
                    vt = vt_pool.tile([128, 96], DTR, tag="vt")
                    nc.scalar.copy(vt[:sz, :], pt[:sz, :])
                    nc.tensor.matmul(
                        gps[t][:], vt[:sz, :], vt[:sz, :],
                        start=(c == 0), stop=(c == 4),
                    )
            for t in (1, 0):
                nc.scalar.copy(g_sb[:, t * 96 : (t + 1) * 96], gps[t][:])
                nc.gpsimd.dma_start(
                    out=g_out[t], in_=g_sb[:, t * 96 : (t + 1) * 96]
                )

    nc.finalize()
    return nc


def _get_nc():
    if "nc" not in _STATE:
        _STATE["nc"] = _build_nc()
    return _STATE["nc"]


def _prep_weights(W1, W2, W3):
    # wt[m, dx, dy*32+ic, oc] = W_m[oc, ic, dy, dx] / 64   (pool-mean folded in)
    wt = np.stack(
        [np.asarray(w, np.float64).transpose(3, 2, 1, 0).reshape(3, 96, 32)
         for w in (W1, W2, W3)]
    ) / 64.0
    return np.ascontiguousarray(wt, dtype=np.float32)


def _host_loss(G):
    G = np.asarray(G, np.float64)  # [16, 96, 96]
    T = G.shape[0]
    I96 = np.eye(M)
    Me = I96[None] + ALPHA_E * G
    ld_e = 2.0 * np.log(
        np.diagonal(np.linalg.cholesky(Me), axis1=-2, axis2=-1)
    ).sum()
    blocks = np.stack(
        [G[:, 32 * c : 32 * (c + 1), 32 * c : 32 * (c + 1)] for c in range(3)]
    )  # [3, T, 32, 32]
    Mc = np.eye(32)[None, None] + ALPHA_C * blocks
    ld_c = 2.0 * np.log(
        np.diagonal(np.linalg.cholesky(Mc), axis1=-2, axis2=-1)
    ).sum()
    loss_expd = ld_e / (2.0 * T)
    loss_comp = (32.0 / M) * ld_c / (2.0 * T)
    return np.float32(loss_expd - loss_comp)


def run_device(inputs, **kw):
    """Run the bass kernel; returns (G [16,96,96], BassKernelResults)."""
    from concourse.bass_utils import run_bass_kernel_spmd

    nc = _get_nc()
    wt = _prep_weights(inputs["W1"], inputs["W2"], inputs["W3"])
    ident = np.eye(128, dtype=np.float32)
    ms = np.asarray(inputs["ms_fea"], np.float32)
    pan = np.asarray(inputs["pan_fea"], np.float32)
    alf = np.asarray(inputs["all_fea"], np.float32)
    in_maps = []
    for i in range(NCORES):
        sl = slice(TPC * i, TPC * (i + 1))
        # x[t*3+m] = (ms,pan,alf)[m][t]
        xs = np.stack([ms[sl], pan[sl], alf[sl]], axis=1).reshape(
            TPC * 3, CCH, H, W
        )
        xa = np.ascontiguousarray(xs[0:4])
        # xb[k*64+g'*32+c, 24q+r, w] = xs[4+g'][c, 48q+24k+r, w]
        xbv = xs[4:6].reshape(2, CCH, 4, 2, 24, W)  # [g', c, q, k, r, w]
        xbv = xbv.transpose(3, 0, 1, 2, 4, 5).reshape(128, 96, W)
        in_maps.append(
            {
                "xa": xa,
                "xb": np.ascontiguousarray(xbv),
                "wt": wt,
                "ident": ident,
            }
        )
    res = run_bass_kernel_spmd(nc, in_maps, core_ids=list(range(NCORES)), **kw)
    G = np.concatenate([np.asarray(r["g_out"]) for r in res.results], axis=0)
    return G, res


def kernel(**inputs):
    G, _ = run_device(inputs)
    return _host_loss(G)
